# revision 8
# baseline (speedup 1.0000x reference)
"""Causal self-attention (QKV proj + RoPE + causal SDPA + out proj) on 8 trn2 cores.

Sharding: tensor-parallel over heads. Each core owns 2 of 16 heads:
  - Wqkv column-split (the core's q/k/v head rows), Wproj row-split.
  - Each core computes a full-shape partial of the output projection;
    the 8 partials are summed (and transposed back) on the host.

Device-side layout trick: everything runs transposed. The host feeds
x^T [C, B*T]; the QKV projection computes qkv^T = Wslice @ x with the
head dim on partitions, which is exactly what Q@K^T and the output
projection want as inputs, so no on-chip transposes are needed except
V (done with DMA xbar transposes, off the critical engines).
"""
import sys

sys.path.insert(0, "/opt/trn_rl_repo")

import numpy as np
import ml_dtypes

import concourse.bacc as bacc
import concourse.mybir as mybir
import concourse.tile as tile
from concourse.bass_utils import run_bass_kernel_spmd

N_CORES = 8
C = 2048
H = 16
D = 128
HPC = H // N_CORES          # heads per core = 2
PB = 512                    # row panel width
JB = 128                    # key tile width
NEG = -1.0e30
ROPE_BASE = 10000.0

BF = mybir.dt.bfloat16
F32 = mybir.dt.float32


def build_module(B, T):
    BT = B * T
    CC = C // 128            # contraction chunks for the projection
    FT = 3 * HPC             # qkv f-tiles per core (q0 q1 k0 k1 v0 v1)
    NPB = T // PB            # panels per batch
    NOC = C // 128           # out-proj column tiles
    scale = 1.0 / float(np.sqrt(D))

    nc = bacc.Bacc("TRN2", target_bir_lowering=False, debug=False,
                   num_devices=N_CORES)

    xT = nc.dram_tensor("xT", [C, BT], BF, kind="ExternalInput").ap()
    wqkvT = nc.dram_tensor("wqkvT", [C, FT * 128], BF, kind="ExternalInput").ap()
    wprojT = nc.dram_tensor("wprojT", [HPC * 128, C], BF, kind="ExternalInput").ap()
    cosT = nc.dram_tensor("cosT", [128, T], BF, kind="ExternalInput").ap()
    sinT = nc.dram_tensor("sinT", [128, T], F32, kind="ExternalInput").ap()
    maskT = nc.dram_tensor("maskT", [128, 896], F32, kind="ExternalInput").ap()
    permT = nc.dram_tensor("permT", [128, 128], BF, kind="ExternalInput").ap()
    zout = nc.dram_tensor("zout", [C, BT], F32, kind="ExternalOutput").ap()

    with tile.TileContext(nc) as tc:
        with tc.tile_pool(name="sb", bufs=1) as sb, \
             tc.tile_pool(name="ps", bufs=1, space="PSUM") as ps:
            # ---- resident constants ----
            wqkv_sb = sb.tile([128, CC, FT * 128], BF, tag="wqkv", bufs=1)
            nc.sync.dma_start(
                out=wqkv_sb[:],
                in_=wqkvT.rearrange("(cc p) f -> p cc f", p=128))
            wproj_sb = sb.tile([128, HPC, C], BF, tag="wproj", bufs=1)
            nc.sync.dma_start(
                out=wproj_sb[:],
                in_=wprojT.rearrange("(hh p) o -> p hh o", p=128))
            cos_sb = sb.tile([128, T], BF, tag="cos", bufs=1)
            nc.sync.dma_start(out=cos_sb[:], in_=cosT)
            sin_sb = sb.tile([128, T], F32, tag="sin", bufs=1)
            nc.sync.dma_start(out=sin_sb[:], in_=sinT)
            mask_sb = sb.tile([128, 896], F32, tag="mask", bufs=1)
            nc.sync.dma_start(out=mask_sb[:], in_=maskT)
            perm_sb = sb.tile([128, 128], BF, tag="perm", bufs=1)
            nc.sync.dma_start(out=perm_sb[:], in_=permT)
            ones_col = sb.tile([128, 1], BF, tag="ones_c", bufs=1)
            nc.vector.memset(ones_col[:], 1.0)
            ones_row = sb.tile([1, 128], BF, tag="ones_r", bufs=1)
            nc.vector.memset(ones_row[:], 1.0)

            xT_r = xT.rearrange("(cc p) r -> p cc r", p=128)

            def emit_outproj(ypair, b, pp):
                r0g = b * T + pp * PB
                for oc in range(NOC):
                    zps = ps.tile([128, PB], F32, tag="mm", bufs=4)
                    for hh in range(HPC):
                        nc.tensor.matmul(
                            zps[:],
                            lhsT=wproj_sb[:, hh, oc * 128:(oc + 1) * 128],
                            rhs=ypair[hh][:],
                            start=(hh == 0), stop=(hh == HPC - 1))
                    zst = sb.tile([128, PB], F32, tag="zst", bufs=3)
                    nc.vector.tensor_copy(out=zst[:], in_=zps[:])
                    nc.sync.dma_start(
                        out=zout[oc * 128:(oc + 1) * 128, r0g:r0g + PB],
                        in_=zst[:])

            def load_xt(b, pp):
                r0g = b * T + pp * PB
                xt = sb.tile([128, CC, PB], BF, tag="xt", bufs=2,
                             name=f"xt_{b}_{pp}")
                for cc in range(CC):
                    nc.gpsimd.dma_start(out=xt[:, cc, :],
                                        in_=xT_r[:, cc, r0g:r0g + PB])
                return xt

            pending = None
            xt_cur = load_xt(0, 0)
            for b in range(B):
                # ---------- projection + rope for batch b ----------
                q_t = [sb.tile([128, T], BF, tag=f"q{h}", bufs=2,
                               name=f"q{h}_{b}")
                       for h in range(HPC)]
                k_t = [sb.tile([128, T], BF, tag=f"k{h}", bufs=2,
                               name=f"k{h}_{b}")
                       for h in range(HPC)]
                v_t = [sb.tile([128, T // 128, 128], BF, tag=f"v{h}", bufs=2,
                               name=f"v{h}_{b}")
                       for h in range(HPC)]
                for pp in range(NPB):
                    r0g = b * T + pp * PB
                    ts = slice(pp * PB, pp * PB + PB)
                    xt = xt_cur
                    nb, npp = (b, pp + 1) if pp + 1 < NPB else (b + 1, 0)
                    xt_cur = load_xt(nb, npp) if nb < B else None
                    for ft in range(FT):
                        pps = ps.tile([128, PB], F32, tag="mm", bufs=4)
                        for cc in range(CC):
                            nc.tensor.matmul(
                                pps[:],
                                lhsT=wqkv_sb[:, cc, ft * 128:(ft + 1) * 128],
                                rhs=xt[:, cc, :],
                                start=(cc == 0), stop=(cc == CC - 1))
                        if ft < 2 * HPC:   # q or k: apply rope
                            raw = sb.tile([128, PB], BF, tag="qkraw", bufs=2)
                            nc.scalar.copy(out=raw[:], in_=pps[:])
                            rot = ps.tile([128, PB], F32, tag="mm", bufs=4)
                            nc.tensor.matmul(rot[:], lhsT=perm_sb[:],
                                             rhs=raw[:], start=True, stop=True)
                            t1 = sb.tile([128, PB], F32, tag="t1", bufs=2)
                            nc.vector.tensor_mul(out=t1[:], in0=raw[:],
                                                 in1=cos_sb[:, ts])
                            t2 = sb.tile([128, PB], F32, tag="t2", bufs=2)
                            nc.vector.tensor_mul(out=t2[:], in0=rot[:],
                                                 in1=sin_sb[:, ts])
                            dest = (q_t if ft < HPC else k_t)[ft % HPC]
                            nc.vector.tensor_add(out=dest[:, ts], in0=t1[:],
                                                 in1=t2[:])
                        else:              # v: stage + dma-transpose
                            h = ft - 2 * HPC
                            vst = sb.tile([128, PB], BF, tag="vstage", bufs=2)
                            nc.scalar.copy(out=vst[:], in_=pps[:])
                            teng = nc.sync if h == 0 else nc.scalar
                            for q4 in range(PB // 128):
                                jt = pp * (PB // 128) + q4
                                teng.dma_start_transpose(
                                    out=v_t[h][:, jt, :],
                                    in_=vst[:, q4 * 128:(q4 + 1) * 128])
                    if pp == 0 and pending is not None:
                        emit_outproj(*pending)
                        pending = None
                # ---------- attention + out-proj for batch b ----------
                for pp in range(NPB):
                    nj = (pp + 1) * (PB // JB)
                    q0 = pp * PB
                    ytil = [ps.tile([128, PB], F32, tag="ytil", bufs=2,
                                    name=f"ytil{h}_{b}_{pp}")
                            for h in range(HPC)]
                    denom = [ps.tile([1, PB], F32, tag="small", bufs=2,
                                     name=f"den{h}_{b}_{pp}")
                             for h in range(HPC)]

                    def emit_S(h, j):
                        kk = j - pp * (PB // JB)
                        lo = max(kk, 0) * 128   # columns < lo fully masked
                        sps = ps.tile([128, PB], F32, tag="mm", bufs=4,
                                      name=f"s{h}_{b}_{pp}_{j}")
                        nc.tensor.matmul(
                            sps[:, lo:PB],
                            lhsT=k_t[h][:, j * JB:(j + 1) * JB],
                            rhs=q_t[h][:, q0 + lo:q0 + PB],
                            start=True, stop=True)
                        return sps

                    def emit_rest(h, j, sps):
                        kk = j - pp * (PB // JB)
                        lo = max(kk, 0) * 128
                        e = sb.tile([128, PB], BF, tag="e", bufs=4,
                                    name=f"e{h}_{b}_{pp}_{j}")
                        if kk >= 0:
                            # triangular 128-col slice gets the mask; the
                            # rest of the block is fully valid
                            nc.vector.scalar_tensor_tensor(
                                out=sps[:, lo:lo + 128],
                                in0=sps[:, lo:lo + 128], scalar=scale,
                                in1=mask_sb[:, 384:512],
                                op0=mybir.AluOpType.mult,
                                op1=mybir.AluOpType.add)
                            nc.scalar.activation(
                                out=e[:, lo:lo + 128], in_=sps[:, lo:lo + 128],
                                func=mybir.ActivationFunctionType.Exp)
                            if lo + 128 < PB:
                                nc.scalar.activation(
                                    out=e[:, lo + 128:PB],
                                    in_=sps[:, lo + 128:PB],
                                    func=mybir.ActivationFunctionType.Exp,
                                    scale=scale)
                        else:
                            nc.scalar.activation(
                                out=e[:, lo:PB], in_=sps[:, lo:PB],
                                func=mybir.ActivationFunctionType.Exp,
                                scale=scale)
                        nc.tensor.matmul(denom[h][:, lo:PB], lhsT=ones_col[:],
                                         rhs=e[:, lo:PB], start=(j == 0),
                                         stop=(j == nj - 1))
                        nc.tensor.matmul(ytil[h][:, lo:PB],
                                         lhsT=v_t[h][:, j, :],
                                         rhs=e[:, lo:PB], start=(j == 0),
                                         stop=(j == nj - 1))

                    jobs = [(h, j) for j in range(nj) for h in range(HPC)]
                    spss = {jobs[0]: emit_S(*jobs[0]),
                            jobs[1]: emit_S(*jobs[1])}
                    for idx, (h, j) in enumerate(jobs):
                        if idx + 2 < len(jobs):
                            spss[jobs[idx + 2]] = emit_S(*jobs[idx + 2])
                        emit_rest(h, j, spss.pop((h, j)))

                    ypair = []
                    for h in range(HPC):
                        dbf = sb.tile([1, PB], BF, tag="dbf", bufs=2)
                        nc.scalar.copy(out=dbf[:], in_=denom[h][:])
                        bc = ps.tile([128, PB], F32, tag="small", bufs=2,
                                     name=f"bc{h}_{b}_{pp}")
                        nc.tensor.matmul(bc[:], lhsT=ones_row[:],
                                         rhs=dbf[:], start=True, stop=True)
                        rec = sb.tile([128, PB], F32, tag="rec", bufs=2)
                        nc.vector.reciprocal(out=rec[:], in_=bc[:])
                        yp = sb.tile([128, PB], BF, tag="yp", bufs=6)
                        nc.vector.tensor_mul(out=yp[:], in0=ytil[h][:],
                                             in1=rec[:])
                        ypair.append(yp)
                    if pending is not None:
                        emit_outproj(*pending)
                    pending = (ypair, b, pp)
            emit_outproj(*pending)

    nc.compile()
    return nc


_module_cache = {}


def _get_module(B, T):
    key = (B, T)
    if key not in _module_cache:
        _module_cache[key] = build_module(B, T)
    return _module_cache[key]


def _host_prep(x, Wqkv, Wproj, B, T):
    bf16 = ml_dtypes.bfloat16
    BT = B * T
    xT = np.ascontiguousarray(x.reshape(BT, C).T).astype(bf16)

    inv = 1.0 / (ROPE_BASE ** (np.arange(0, D, 2, dtype=np.float32) / D))
    t = np.arange(T, dtype=np.float32)
    fr = np.outer(t, inv)                      # [T, 64]
    emb = np.concatenate([fr, fr], -1)         # [T, 128]
    cosT = np.ascontiguousarray(np.cos(emb).T).astype(bf16)
    sinT = np.ascontiguousarray(np.sin(emb).T).astype(np.float32)

    g = np.arange(896)[None, :]
    p = np.arange(128)[:, None]
    maskT = np.where(g >= p + 384, 0.0, NEG).astype(np.float32)

    permT = np.zeros((128, 128), np.float32)
    for j in range(64):
        permT[j, j + 64] = 1.0                 # rot[i] = q[i-64] for i>=64
    for j in range(64, 128):
        permT[j, j - 64] = -1.0                # rot[i] = -q[i+64] for i<64
    permT = permT.astype(bf16)

    in_maps = []
    for c in range(N_CORES):
        heads = [HPC * c + h for h in range(HPC)]
        rows = []
        for blk in range(3):                   # q, k, v blocks of Wqkv
            for h in heads:
                r0 = blk * C + h * D
                rows.append(Wqkv[r0:r0 + D])
        wslice = np.concatenate(rows, 0)       # [FT*128, C]
        wqkvT = np.ascontiguousarray(wslice.T).astype(bf16)
        cols = np.concatenate([np.arange(h * D, (h + 1) * D) for h in heads])
        wprojT = np.ascontiguousarray(Wproj[:, cols].T).astype(bf16)
        in_maps.append({
            "xT": xT,
            "wqkvT": wqkvT,
            "wprojT": wprojT,
            "cosT": cosT,
            "sinT": sinT,
            "maskT": maskT,
            "permT": permT,
        })
    return in_maps


last_results = None


def kernel(x, Wqkv, Wproj, _trace=False, _trace_kwargs=None):
    global last_results
    x = np.asarray(x, dtype=np.float32)
    Wqkv = np.asarray(Wqkv, dtype=np.float32)
    Wproj = np.asarray(Wproj, dtype=np.float32)
    B, T, _C = x.shape
    assert _C == C and T % PB == 0

    nc = _get_module(B, T)
    in_maps = _host_prep(x, Wqkv, Wproj, B, T)
    res = run_bass_kernel_spmd(nc, in_maps, core_ids=list(range(N_CORES)),
                               trace=_trace, **(_trace_kwargs or {}))
    last_results = res
    z = res.results[0]["zout"].astype(np.float32)
    for c in range(1, N_CORES):
        z += res.results[c]["zout"]
    y = np.ascontiguousarray(z.T).reshape(B, T, C)
    return y


# revision 11
# speedup vs baseline: 1.1121x; 1.1121x over previous
"""Causal self-attention (QKV proj + RoPE + causal SDPA + out proj) on 8 trn2 cores.

Sharding: tensor-parallel over heads. Each core owns 2 of 16 heads:
  - Wqkv column-split (the core's q/k/v head rows), Wproj row-split.
  - Each core computes a full-shape partial of the output projection;
    the 8 partials are summed (and transposed back) on the host.

Device-side layout trick: everything runs transposed. The host feeds
x^T [C, B*T]; the QKV projection computes qkv^T = Wslice @ x with the
head dim on partitions, which is exactly what Q@K^T and the output
projection want as inputs, so no on-chip transposes are needed except
V (done with DMA xbar transposes, off the critical engines).
"""
import sys

sys.path.insert(0, "/opt/trn_rl_repo")

import numpy as np
import ml_dtypes

import concourse.bacc as bacc
import concourse.mybir as mybir
import concourse.tile as tile
from concourse.bass_utils import run_bass_kernel_spmd

N_CORES = 8
C = 2048
H = 16
D = 128
HPC = H // N_CORES          # heads per core = 2
PB = 512                    # row panel width
JB = 128                    # key tile width
NEG = -1.0e30
ROPE_BASE = 10000.0

BF = mybir.dt.bfloat16
F32 = mybir.dt.float32


def build_module(B, T):
    BT = B * T
    CC = C // 128            # contraction chunks for the projection
    FT = 3 * HPC             # qkv f-tiles per core (q0 q1 k0 k1 v0 v1)
    NPB = T // PB            # panels per batch
    NOC = C // 128           # out-proj column tiles
    scale = 1.0 / float(np.sqrt(D))

    nc = bacc.Bacc("TRN2", target_bir_lowering=False, debug=False,
                   num_devices=N_CORES)

    xT = nc.dram_tensor("xT", [C, BT], BF, kind="ExternalInput").ap()
    wqkvT = nc.dram_tensor("wqkvT", [C, FT * 128], BF, kind="ExternalInput").ap()
    wprojT = nc.dram_tensor("wprojT", [HPC * 128, C], BF, kind="ExternalInput").ap()
    cosT = nc.dram_tensor("cosT", [128, T], BF, kind="ExternalInput").ap()
    sinT = nc.dram_tensor("sinT", [128, T], F32, kind="ExternalInput").ap()
    maskT = nc.dram_tensor("maskT", [128, 896], F32, kind="ExternalInput").ap()
    permT = nc.dram_tensor("permT", [128, 128], BF, kind="ExternalInput").ap()
    zout = nc.dram_tensor("zout", [C, BT], F32, kind="ExternalOutput").ap()

    with tile.TileContext(nc) as tc:
        with tc.tile_pool(name="sb", bufs=1) as sb, \
             tc.tile_pool(name="ps", bufs=1, space="PSUM") as ps:
            # ---- resident constants ----
            wqkv_sb = sb.tile([128, CC, FT * 128], BF, tag="wqkv", bufs=1)
            nc.sync.dma_start(
                out=wqkv_sb[:],
                in_=wqkvT.rearrange("(cc p) f -> p cc f", p=128))
            wproj_sb = sb.tile([128, HPC, C], BF, tag="wproj", bufs=1)
            nc.sync.dma_start(
                out=wproj_sb[:],
                in_=wprojT.rearrange("(hh p) o -> p hh o", p=128))
            cos_sb = sb.tile([128, T], BF, tag="cos", bufs=1)
            nc.sync.dma_start(out=cos_sb[:], in_=cosT)
            sin_sb = sb.tile([128, T], F32, tag="sin", bufs=1)
            nc.sync.dma_start(out=sin_sb[:], in_=sinT)
            mask_sb = sb.tile([128, 896], F32, tag="mask", bufs=1)
            nc.sync.dma_start(out=mask_sb[:], in_=maskT)
            perm_sb = sb.tile([128, 128], BF, tag="perm", bufs=1)
            nc.sync.dma_start(out=perm_sb[:], in_=permT)
            ones_col = sb.tile([128, 1], BF, tag="ones_c", bufs=1)
            nc.vector.memset(ones_col[:], 1.0)
            ones_row = sb.tile([1, 128], BF, tag="ones_r", bufs=1)
            nc.vector.memset(ones_row[:], 1.0)

            xT_r = xT.rearrange("(cc p) r -> p cc r", p=128)

            def emit_outproj(ypair, b, pp):
                r0g = b * T + pp * PB
                for oc in range(NOC):
                    zps = ps.tile([128, PB], F32, tag="mm", bufs=4)
                    for hh in range(HPC):
                        nc.tensor.matmul(
                            zps[:],
                            lhsT=wproj_sb[:, hh, oc * 128:(oc + 1) * 128],
                            rhs=ypair[hh][:],
                            start=(hh == 0), stop=(hh == HPC - 1))
                    zst = sb.tile([128, PB], F32, tag="zst", bufs=4)
                    nc.vector.tensor_copy(out=zst[:], in_=zps[:])
                    nc.gpsimd.dma_start(
                        out=zout[oc * 128:(oc + 1) * 128, r0g:r0g + PB],
                        in_=zst[:])

            def load_xt(b, pp):
                r0g = b * T + pp * PB
                xt = sb.tile([128, CC, PB], BF, tag="xt", bufs=2,
                             name=f"xt_{b}_{pp}")
                for cc in range(CC):
                    nc.sync.dma_start(out=xt[:, cc, :],
                                      in_=xT_r[:, cc, r0g:r0g + PB])
                return xt

            pending = None
            xt_cur = load_xt(0, 0)
            for b in range(B):
                # ---------- projection + rope for batch b ----------
                q_t = [sb.tile([128, T], BF, tag=f"q{h}", bufs=2,
                               name=f"q{h}_{b}")
                       for h in range(HPC)]
                k_t = [sb.tile([128, T], BF, tag=f"k{h}", bufs=2,
                               name=f"k{h}_{b}")
                       for h in range(HPC)]
                v_t = [sb.tile([128, T // 128, 128], BF, tag=f"v{h}", bufs=2,
                               name=f"v{h}_{b}")
                       for h in range(HPC)]
                for pp in range(NPB):
                    r0g = b * T + pp * PB
                    ts = slice(pp * PB, pp * PB + PB)
                    xt = xt_cur
                    nb, npp = (b, pp + 1) if pp + 1 < NPB else (b + 1, 0)
                    xt_cur = load_xt(nb, npp) if nb < B else None
                    for ft in range(FT):
                        pps = ps.tile([128, PB], F32, tag="mm", bufs=4)
                        for cc in range(CC):
                            nc.tensor.matmul(
                                pps[:],
                                lhsT=wqkv_sb[:, cc, ft * 128:(ft + 1) * 128],
                                rhs=xt[:, cc, :],
                                start=(cc == 0), stop=(cc == CC - 1))
                        if ft < 2 * HPC:   # q or k: apply rope
                            raw = sb.tile([128, PB], BF, tag="qkraw", bufs=2)
                            nc.scalar.copy(out=raw[:], in_=pps[:])
                            rot = ps.tile([128, PB], F32, tag="mm", bufs=4)
                            nc.tensor.matmul(rot[:], lhsT=perm_sb[:],
                                             rhs=raw[:], start=True, stop=True)
                            t1 = sb.tile([128, PB], F32, tag="t1", bufs=2)
                            nc.vector.tensor_mul(out=t1[:], in0=raw[:],
                                                 in1=cos_sb[:, ts])
                            t2 = sb.tile([128, PB], F32, tag="t2", bufs=2)
                            nc.vector.tensor_mul(out=t2[:], in0=rot[:],
                                                 in1=sin_sb[:, ts])
                            dest = (q_t if ft < HPC else k_t)[ft % HPC]
                            nc.vector.tensor_add(out=dest[:, ts], in0=t1[:],
                                                 in1=t2[:])
                        else:              # v: stage + dma-transpose
                            h = ft - 2 * HPC
                            vst = sb.tile([128, PB], BF, tag="vstage", bufs=2)
                            nc.scalar.copy(out=vst[:], in_=pps[:])
                            teng = nc.sync if h == 0 else nc.scalar
                            for q4 in range(PB // 128):
                                jt = pp * (PB // 128) + q4
                                teng.dma_start_transpose(
                                    out=v_t[h][:, jt, :],
                                    in_=vst[:, q4 * 128:(q4 + 1) * 128])
                    if pp == 0 and pending is not None:
                        emit_outproj(*pending)
                        pending = None
                # ---------- attention + out-proj for batch b ----------
                for pp in range(NPB):
                    nj = (pp + 1) * (PB // JB)
                    q0 = pp * PB
                    ytil = [ps.tile([128, PB], F32, tag="ytil", bufs=2,
                                    name=f"ytil{h}_{b}_{pp}")
                            for h in range(HPC)]
                    denom = [ps.tile([1, PB], F32, tag="small", bufs=2,
                                     name=f"den{h}_{b}_{pp}")
                             for h in range(HPC)]

                    def emit_S(h, j):
                        kk = j - pp * (PB // JB)
                        lo = max(kk, 0) * 128   # columns < lo fully masked
                        sps = ps.tile([128, PB], F32, tag="mm", bufs=4,
                                      name=f"s{h}_{b}_{pp}_{j}")
                        nc.tensor.matmul(
                            sps[:, lo:PB],
                            lhsT=k_t[h][:, j * JB:(j + 1) * JB],
                            rhs=q_t[h][:, q0 + lo:q0 + PB],
                            start=True, stop=True)
                        return sps

                    def emit_rest(h, j, sps):
                        kk = j - pp * (PB // JB)
                        lo = max(kk, 0) * 128
                        e = sb.tile([128, PB], BF, tag="e", bufs=4,
                                    name=f"e{h}_{b}_{pp}_{j}")
                        if kk >= 0:
                            # triangular 128-col slice gets the mask; the
                            # rest of the block is fully valid
                            nc.vector.scalar_tensor_tensor(
                                out=sps[:, lo:lo + 128],
                                in0=sps[:, lo:lo + 128], scalar=scale,
                                in1=mask_sb[:, 384:512],
                                op0=mybir.AluOpType.mult,
                                op1=mybir.AluOpType.add)
                            nc.scalar.activation(
                                out=e[:, lo:lo + 128], in_=sps[:, lo:lo + 128],
                                func=mybir.ActivationFunctionType.Exp)
                            if lo + 128 < PB:
                                nc.scalar.activation(
                                    out=e[:, lo + 128:PB],
                                    in_=sps[:, lo + 128:PB],
                                    func=mybir.ActivationFunctionType.Exp,
                                    scale=scale)
                        else:
                            nc.scalar.activation(
                                out=e[:, lo:PB], in_=sps[:, lo:PB],
                                func=mybir.ActivationFunctionType.Exp,
                                scale=scale)
                        nc.tensor.matmul(denom[h][:, lo:PB], lhsT=ones_col[:],
                                         rhs=e[:, lo:PB], start=(j == 0),
                                         stop=(j == nj - 1))
                        nc.tensor.matmul(ytil[h][:, lo:PB],
                                         lhsT=v_t[h][:, j, :],
                                         rhs=e[:, lo:PB], start=(j == 0),
                                         stop=(j == nj - 1))

                    jobs = [(h, j) for j in range(nj) for h in range(HPC)]
                    spss = {jobs[0]: emit_S(*jobs[0]),
                            jobs[1]: emit_S(*jobs[1])}
                    for idx, (h, j) in enumerate(jobs):
                        if idx + 2 < len(jobs):
                            spss[jobs[idx + 2]] = emit_S(*jobs[idx + 2])
                        emit_rest(h, j, spss.pop((h, j)))

                    ypair = []
                    for h in range(HPC):
                        dbf = sb.tile([1, PB], BF, tag="dbf", bufs=2)
                        nc.scalar.copy(out=dbf[:], in_=denom[h][:])
                        bc = ps.tile([128, PB], F32, tag="small", bufs=2,
                                     name=f"bc{h}_{b}_{pp}")
                        nc.tensor.matmul(bc[:], lhsT=ones_row[:],
                                         rhs=dbf[:], start=True, stop=True)
                        rec = sb.tile([128, PB], F32, tag="rec", bufs=2)
                        nc.vector.reciprocal_approx_fast(out=rec[:], in_=bc[:])
                        yp = sb.tile([128, PB], BF, tag="yp", bufs=6)
                        nc.vector.tensor_mul(out=yp[:], in0=ytil[h][:],
                                             in1=rec[:])
                        ypair.append(yp)
                    if pending is not None:
                        emit_outproj(*pending)
                    pending = (ypair, b, pp)
            emit_outproj(*pending)

    nc.compile()
    return nc


_module_cache = {}


def _get_module(B, T):
    key = (B, T)
    if key not in _module_cache:
        _module_cache[key] = build_module(B, T)
    return _module_cache[key]


def _host_prep(x, Wqkv, Wproj, B, T):
    bf16 = ml_dtypes.bfloat16
    BT = B * T
    xT = np.ascontiguousarray(x.reshape(BT, C).T).astype(bf16)

    inv = 1.0 / (ROPE_BASE ** (np.arange(0, D, 2, dtype=np.float32) / D))
    t = np.arange(T, dtype=np.float32)
    fr = np.outer(t, inv)                      # [T, 64]
    emb = np.concatenate([fr, fr], -1)         # [T, 128]
    cosT = np.ascontiguousarray(np.cos(emb).T).astype(bf16)
    sinT = np.ascontiguousarray(np.sin(emb).T).astype(np.float32)

    g = np.arange(896)[None, :]
    p = np.arange(128)[:, None]
    maskT = np.where(g >= p + 384, 0.0, NEG).astype(np.float32)

    permT = np.zeros((128, 128), np.float32)
    for j in range(64):
        permT[j, j + 64] = 1.0                 # rot[i] = q[i-64] for i>=64
    for j in range(64, 128):
        permT[j, j - 64] = -1.0                # rot[i] = -q[i+64] for i<64
    permT = permT.astype(bf16)

    in_maps = []
    for c in range(N_CORES):
        heads = [HPC * c + h for h in range(HPC)]
        rows = []
        for blk in range(3):                   # q, k, v blocks of Wqkv
            for h in heads:
                r0 = blk * C + h * D
                rows.append(Wqkv[r0:r0 + D])
        wslice = np.concatenate(rows, 0)       # [FT*128, C]
        wqkvT = np.ascontiguousarray(wslice.T).astype(bf16)
        cols = np.concatenate([np.arange(h * D, (h + 1) * D) for h in heads])
        wprojT = np.ascontiguousarray(Wproj[:, cols].T).astype(bf16)
        in_maps.append({
            "xT": xT,
            "wqkvT": wqkvT,
            "wprojT": wprojT,
            "cosT": cosT,
            "sinT": sinT,
            "maskT": maskT,
            "permT": permT,
        })
    return in_maps


last_results = None


def kernel(x, Wqkv, Wproj, _trace=False, _trace_kwargs=None):
    global last_results
    x = np.asarray(x, dtype=np.float32)
    Wqkv = np.asarray(Wqkv, dtype=np.float32)
    Wproj = np.asarray(Wproj, dtype=np.float32)
    B, T, _C = x.shape
    assert _C == C and T % PB == 0

    nc = _get_module(B, T)
    in_maps = _host_prep(x, Wqkv, Wproj, B, T)
    res = run_bass_kernel_spmd(nc, in_maps, core_ids=list(range(N_CORES)),
                               trace=_trace, **(_trace_kwargs or {}))
    last_results = res
    z = res.results[0]["zout"].astype(np.float32)
    for c in range(1, N_CORES):
        z += res.results[c]["zout"]
    y = np.ascontiguousarray(z.T).reshape(B, T, C)
    return y


# revision 13
# speedup vs baseline: 1.1514x; 1.0354x over previous
"""Causal self-attention (QKV proj + RoPE + causal SDPA + out proj) on 8 trn2 cores.

Sharding: tensor-parallel over heads. Each core owns 2 of 16 heads:
  - Wqkv column-split (the core's q/k/v head rows), Wproj row-split.
  - Each core computes a full-shape partial of the output projection;
    the 8 partials are summed (and transposed back) on the host.

Device-side layout trick: everything runs transposed. The host feeds
x^T [C, B*T]; the QKV projection computes qkv^T = Wslice @ x with the
head dim on partitions, which is exactly what Q@K^T and the output
projection want as inputs, so no on-chip transposes are needed except
V (done with DMA xbar transposes, off the critical engines).
"""
import sys

sys.path.insert(0, "/opt/trn_rl_repo")

import numpy as np
import ml_dtypes

import concourse.bacc as bacc
import concourse.mybir as mybir
import concourse.tile as tile
from concourse.bass_utils import run_bass_kernel_spmd

N_CORES = 8
C = 2048
H = 16
D = 128
HPC = H // N_CORES          # heads per core = 2
PB = 512                    # row panel width
JB = 128                    # key tile width
NEG = -1.0e30
ROPE_BASE = 10000.0

BF = mybir.dt.bfloat16
F32 = mybir.dt.float32


def build_module(B, T):
    BT = B * T
    CC = C // 128            # contraction chunks for the projection
    FT = 3 * HPC             # qkv f-tiles per core (q0 q1 k0 k1 v0 v1)
    NPB = T // PB            # panels per batch
    NOC = C // 128           # out-proj column tiles
    scale = 1.0 / float(np.sqrt(D))

    nc = bacc.Bacc("TRN2", target_bir_lowering=False, debug=False,
                   num_devices=N_CORES)

    xT = nc.dram_tensor("xT", [C, BT], BF, kind="ExternalInput").ap()
    wqkvT = nc.dram_tensor("wqkvT", [C, FT * 128], BF, kind="ExternalInput").ap()
    wprojT = nc.dram_tensor("wprojT", [HPC * 128, C], BF, kind="ExternalInput").ap()
    cosT = nc.dram_tensor("cosT", [128, T], BF, kind="ExternalInput").ap()
    sinT = nc.dram_tensor("sinT", [128, T], F32, kind="ExternalInput").ap()
    maskT = nc.dram_tensor("maskT", [128, 896], F32, kind="ExternalInput").ap()
    permT = nc.dram_tensor("permT", [128, 128], BF, kind="ExternalInput").ap()
    zout = nc.dram_tensor("zout", [C, BT], F32, kind="ExternalOutput").ap()

    with tile.TileContext(nc) as tc:
        with tc.tile_pool(name="sb", bufs=1) as sb, \
             tc.tile_pool(name="ps", bufs=1, space="PSUM") as ps:
            # ---- resident constants ----
            wqkv_sb = sb.tile([128, CC, FT * 128], BF, tag="wqkv", bufs=1)
            nc.sync.dma_start(
                out=wqkv_sb[:],
                in_=wqkvT.rearrange("(cc p) f -> p cc f", p=128))
            wproj_sb = sb.tile([128, HPC, C], BF, tag="wproj", bufs=1)
            nc.sync.dma_start(
                out=wproj_sb[:],
                in_=wprojT.rearrange("(hh p) o -> p hh o", p=128))
            cos_sb = sb.tile([128, T], BF, tag="cos", bufs=1)
            nc.sync.dma_start(out=cos_sb[:], in_=cosT)
            sin_sb = sb.tile([128, T], F32, tag="sin", bufs=1)
            nc.sync.dma_start(out=sin_sb[:], in_=sinT)
            mask_sb = sb.tile([128, 896], F32, tag="mask", bufs=1)
            nc.sync.dma_start(out=mask_sb[:], in_=maskT)
            perm_sb = sb.tile([128, 128], BF, tag="perm", bufs=1)
            nc.sync.dma_start(out=perm_sb[:], in_=permT)
            ones_col = sb.tile([128, 1], BF, tag="ones_c", bufs=1)
            nc.vector.memset(ones_col[:], 1.0)
            ones_row = sb.tile([1, 128], BF, tag="ones_r", bufs=1)
            nc.vector.memset(ones_row[:], 1.0)

            xT_r = xT.rearrange("(cc p) r -> p cc r", p=128)

            def emit_outproj(ypair, b, pp):
                r0g = b * T + pp * PB
                for oc in range(NOC):
                    zps = ps.tile([128, PB], F32, tag="mm", bufs=4)
                    for hh in range(HPC):
                        nc.tensor.matmul(
                            zps[:],
                            lhsT=wproj_sb[:, hh, oc * 128:(oc + 1) * 128],
                            rhs=ypair[hh][:],
                            start=(hh == 0), stop=(hh == HPC - 1))
                    zst = sb.tile([128, PB], F32, tag="zst", bufs=4)
                    nc.vector.tensor_copy(out=zst[:], in_=zps[:])
                    nc.sync.dma_start(
                        out=zout[oc * 128:(oc + 1) * 128, r0g:r0g + PB],
                        in_=zst[:])

            def load_xt(b, pp):
                r0g = b * T + pp * PB
                xt = sb.tile([128, CC, PB], BF, tag="xt", bufs=2,
                             name=f"xt_{b}_{pp}")
                for cc in range(CC):
                    nc.sync.dma_start(out=xt[:, cc, :],
                                      in_=xT_r[:, cc, r0g:r0g + PB])
                return xt

            pending = None
            xt_cur = load_xt(0, 0)
            for b in range(B):
                # ---------- projection + rope for batch b ----------
                q_t = [sb.tile([128, T], BF, tag=f"q{h}", bufs=2,
                               name=f"q{h}_{b}")
                       for h in range(HPC)]
                k_t = [sb.tile([128, T], BF, tag=f"k{h}", bufs=2,
                               name=f"k{h}_{b}")
                       for h in range(HPC)]
                v_t = [sb.tile([128, T // 128, 128], BF, tag=f"v{h}", bufs=2,
                               name=f"v{h}_{b}")
                       for h in range(HPC)]
                for pp in range(NPB):
                    r0g = b * T + pp * PB
                    ts = slice(pp * PB, pp * PB + PB)
                    xt = xt_cur
                    nb, npp = (b, pp + 1) if pp + 1 < NPB else (b + 1, 0)
                    xt_cur = load_xt(nb, npp) if nb < B else None
                    for ft in range(FT):
                        pps = ps.tile([128, PB], F32, tag="mm", bufs=4)
                        for cc in range(CC):
                            nc.tensor.matmul(
                                pps[:],
                                lhsT=wqkv_sb[:, cc, ft * 128:(ft + 1) * 128],
                                rhs=xt[:, cc, :],
                                start=(cc == 0), stop=(cc == CC - 1))
                        if ft < 2 * HPC:   # q or k: apply rope
                            raw = sb.tile([128, PB], BF, tag="qkraw", bufs=2)
                            nc.scalar.copy(out=raw[:], in_=pps[:])
                            rot = ps.tile([128, PB], F32, tag="mm", bufs=4)
                            nc.tensor.matmul(rot[:], lhsT=perm_sb[:],
                                             rhs=raw[:], start=True, stop=True)
                            t1 = sb.tile([128, PB], F32, tag="t1", bufs=2)
                            nc.vector.tensor_mul(out=t1[:], in0=raw[:],
                                                 in1=cos_sb[:, ts])
                            t2 = sb.tile([128, PB], F32, tag="t2", bufs=2)
                            nc.vector.tensor_mul(out=t2[:], in0=rot[:],
                                                 in1=sin_sb[:, ts])
                            dest = (q_t if ft < HPC else k_t)[ft % HPC]
                            nc.vector.tensor_add(out=dest[:, ts], in0=t1[:],
                                                 in1=t2[:])
                        else:              # v: stage + dma-transpose
                            h = ft - 2 * HPC
                            vst = sb.tile([128, PB], BF, tag="vstage", bufs=2)
                            nc.scalar.copy(out=vst[:], in_=pps[:])
                            teng = nc.scalar
                            for q4 in range(PB // 128):
                                jt = pp * (PB // 128) + q4
                                teng.dma_start_transpose(
                                    out=v_t[h][:, jt, :],
                                    in_=vst[:, q4 * 128:(q4 + 1) * 128])
                    if pp == 0 and pending is not None:
                        emit_outproj(*pending)
                        pending = None
                # ---------- attention + out-proj for batch b ----------
                for pp in range(NPB):
                    nj = (pp + 1) * (PB // JB)
                    q0 = pp * PB
                    ytil = [ps.tile([128, PB], F32, tag="ytil", bufs=2,
                                    name=f"ytil{h}_{b}_{pp}")
                            for h in range(HPC)]
                    denom = [ps.tile([1, PB], F32, tag="small", bufs=2,
                                     name=f"den{h}_{b}_{pp}")
                             for h in range(HPC)]

                    def emit_S(h, j):
                        kk = j - pp * (PB // JB)
                        lo = max(kk, 0) * 128   # columns < lo fully masked
                        sps = ps.tile([128, PB], F32, tag="mm", bufs=4,
                                      name=f"s{h}_{b}_{pp}_{j}")
                        nc.tensor.matmul(
                            sps[:, lo:PB],
                            lhsT=k_t[h][:, j * JB:(j + 1) * JB],
                            rhs=q_t[h][:, q0 + lo:q0 + PB],
                            start=True, stop=True)
                        return sps

                    def emit_rest(h, j, sps):
                        kk = j - pp * (PB // JB)
                        lo = max(kk, 0) * 128
                        e = sb.tile([128, PB], BF, tag="e", bufs=4,
                                    name=f"e{h}_{b}_{pp}_{j}")
                        if kk >= 0:
                            # triangular 128-col slice gets the mask; the
                            # rest of the block is fully valid
                            nc.vector.scalar_tensor_tensor(
                                out=sps[:, lo:lo + 128],
                                in0=sps[:, lo:lo + 128], scalar=scale,
                                in1=mask_sb[:, 384:512],
                                op0=mybir.AluOpType.mult,
                                op1=mybir.AluOpType.add)
                            nc.scalar.activation(
                                out=e[:, lo:lo + 128], in_=sps[:, lo:lo + 128],
                                func=mybir.ActivationFunctionType.Exp)
                            if lo + 128 < PB:
                                nc.scalar.activation(
                                    out=e[:, lo + 128:PB],
                                    in_=sps[:, lo + 128:PB],
                                    func=mybir.ActivationFunctionType.Exp,
                                    scale=scale)
                        else:
                            nc.scalar.activation(
                                out=e[:, lo:PB], in_=sps[:, lo:PB],
                                func=mybir.ActivationFunctionType.Exp,
                                scale=scale)
                        nc.tensor.matmul(denom[h][:, lo:PB], lhsT=ones_col[:],
                                         rhs=e[:, lo:PB], start=(j == 0),
                                         stop=(j == nj - 1))
                        nc.tensor.matmul(ytil[h][:, lo:PB],
                                         lhsT=v_t[h][:, j, :],
                                         rhs=e[:, lo:PB], start=(j == 0),
                                         stop=(j == nj - 1))

                    jobs = [(h, j) for j in range(nj) for h in range(HPC)]
                    spss = {jobs[0]: emit_S(*jobs[0]),
                            jobs[1]: emit_S(*jobs[1])}
                    for idx, (h, j) in enumerate(jobs):
                        if idx + 2 < len(jobs):
                            spss[jobs[idx + 2]] = emit_S(*jobs[idx + 2])
                        emit_rest(h, j, spss.pop((h, j)))

                    ypair = []
                    for h in range(HPC):
                        dbf = sb.tile([1, PB], BF, tag="dbf", bufs=2)
                        nc.scalar.copy(out=dbf[:], in_=denom[h][:])
                        bc = ps.tile([128, PB], F32, tag="small", bufs=2,
                                     name=f"bc{h}_{b}_{pp}")
                        nc.tensor.matmul(bc[:], lhsT=ones_row[:],
                                         rhs=dbf[:], start=True, stop=True)
                        rec = sb.tile([128, PB], F32, tag="rec", bufs=2)
                        nc.vector.reciprocal_approx_fast(out=rec[:], in_=bc[:])
                        yp = sb.tile([128, PB], BF, tag="yp", bufs=6)
                        nc.vector.tensor_mul(out=yp[:], in0=ytil[h][:],
                                             in1=rec[:])
                        ypair.append(yp)
                    if pending is not None:
                        emit_outproj(*pending)
                    pending = (ypair, b, pp)
            emit_outproj(*pending)

    nc.compile()
    return nc


_module_cache = {}


def _get_module(B, T):
    key = (B, T)
    if key not in _module_cache:
        _module_cache[key] = build_module(B, T)
    return _module_cache[key]


def _host_prep(x, Wqkv, Wproj, B, T):
    bf16 = ml_dtypes.bfloat16
    BT = B * T
    xT = np.ascontiguousarray(x.reshape(BT, C).T).astype(bf16)

    inv = 1.0 / (ROPE_BASE ** (np.arange(0, D, 2, dtype=np.float32) / D))
    t = np.arange(T, dtype=np.float32)
    fr = np.outer(t, inv)                      # [T, 64]
    emb = np.concatenate([fr, fr], -1)         # [T, 128]
    cosT = np.ascontiguousarray(np.cos(emb).T).astype(bf16)
    sinT = np.ascontiguousarray(np.sin(emb).T).astype(np.float32)

    g = np.arange(896)[None, :]
    p = np.arange(128)[:, None]
    maskT = np.where(g >= p + 384, 0.0, NEG).astype(np.float32)

    permT = np.zeros((128, 128), np.float32)
    for j in range(64):
        permT[j, j + 64] = 1.0                 # rot[i] = q[i-64] for i>=64
    for j in range(64, 128):
        permT[j, j - 64] = -1.0                # rot[i] = -q[i+64] for i<64
    permT = permT.astype(bf16)

    in_maps = []
    for c in range(N_CORES):
        heads = [HPC * c + h for h in range(HPC)]
        rows = []
        for blk in range(3):                   # q, k, v blocks of Wqkv
            for h in heads:
                r0 = blk * C + h * D
                rows.append(Wqkv[r0:r0 + D])
        wslice = np.concatenate(rows, 0)       # [FT*128, C]
        wqkvT = np.ascontiguousarray(wslice.T).astype(bf16)
        cols = np.concatenate([np.arange(h * D, (h + 1) * D) for h in heads])
        wprojT = np.ascontiguousarray(Wproj[:, cols].T).astype(bf16)
        in_maps.append({
            "xT": xT,
            "wqkvT": wqkvT,
            "wprojT": wprojT,
            "cosT": cosT,
            "sinT": sinT,
            "maskT": maskT,
            "permT": permT,
        })
    return in_maps


last_results = None


def kernel(x, Wqkv, Wproj, _trace=False, _trace_kwargs=None):
    global last_results
    x = np.asarray(x, dtype=np.float32)
    Wqkv = np.asarray(Wqkv, dtype=np.float32)
    Wproj = np.asarray(Wproj, dtype=np.float32)
    B, T, _C = x.shape
    assert _C == C and T % PB == 0

    nc = _get_module(B, T)
    in_maps = _host_prep(x, Wqkv, Wproj, B, T)
    res = run_bass_kernel_spmd(nc, in_maps, core_ids=list(range(N_CORES)),
                               trace=_trace, **(_trace_kwargs or {}))
    last_results = res
    z = res.results[0]["zout"].astype(np.float32)
    for c in range(1, N_CORES):
        z += res.results[c]["zout"]
    y = np.ascontiguousarray(z.T).reshape(B, T, C)
    return y
